# revision 32
# baseline (speedup 1.0000x reference)
"""Trainium2 Bass kernel for the DLEM converter + diagonal-update model.

Per batch:
    h1 = relu(conv1d(signal[128ch -> 10ch], k=3))        # [10, 8190]
    h2 = relu(conv1d(h1, k=1))                           # [10, 8190]
    h3 = relu(conv_transpose1d(h2, k=3))                 # [10, 8192]
    lr = sigmoid(conv1d(h3[10ch -> 2ch], k=1))           # [2, 8192]
    mass_in  = cd[1:]*right[1:n-1] + cd[:-1]*left[1:n-1]
    mass_out = right[0:n-2] + left[2:n]
    nd = ln(const*mass_in) - ln(mass_out);  out = nd - mean(nd)

Sharding: data-parallel over batch, 4 batches per core on 8 cores.

Device layout: the 4 local batches live in partition strips at 0/32/64/96
(matmul col-groups are 32-aligned), so conv2/convT/conv3 are single
block-diagonal fp32r matmuls (K=128) covering all 4 batches, and conv1 is
12 accumulating fp32r matmuls per 512-wide tile (4 batches x 3 taps, each
with a zero-padded full-width weight block so every PSUM row is written
and junk partitions accumulate exact zeros).  Biases ride for free on the
ACT/DVE epilogues (relu/sigmoid bias, tensor_scalar add+max).  The
elementwise tail (mass/log) runs in a dense time-partitioned layout
[63, 4*130] reached by bouncing sigmoid rows through a DRAM scratch and
reading back with shifted strided APs, so those ops use 63 lanes instead
of 8.  The global mean subtraction happens on host after the gather.

fp32r (TF32-style) matmuls run at 1 PE column/cycle vs 4 for strict fp32;
measured end-to-end error vs the fp32 reference is ~2e-5.
"""

import numpy as np

N_CORES = 8
B, C, N = 32, 128, 8192
BL = B // N_CORES          # batches per core
ND = N - 2                 # output length per batch (index_diag == 1)
CH = 130                   # time-chunk per partition in the tail layout
PR = 63                    # partitions used in tail (63*130 == 8190)
NT = 16                    # 512-wide time tiles
TW = 512

_prog_cache = {}


def build_program(loop_n=1, sig_batch=True, ps_rebal=True, tail_split=True):
    """Build + compile the per-core Bass program.

    loop_n > 1 wraps the whole body in an on-device For_i loop (used only
    for benchmarking; the work is identical every iteration).
    """
    import concourse.bass as bass
    import concourse.tile as tile
    import concourse.mybir as mybir
    from concourse import bacc
    from contextlib import ExitStack

    f32 = mybir.dt.float32
    f32r = mybir.dt.float32r
    AF = mybir.ActivationFunctionType
    ALU = mybir.AluOpType

    def r(ap):
        return ap.bitcast(f32r)

    nc = bacc.Bacc("TRN2", target_bir_lowering=False, debug=False,
                   num_devices=N_CORES)

    sig = nc.dram_tensor("sig", [BL, C, N], f32, kind="ExternalInput")
    cd0r = nc.dram_tensor("cd0r", [PR, BL * CH], f32, kind="ExternalInput")
    cd1r = nc.dram_tensor("cd1r", [PR, BL * CH], f32, kind="ExternalInput")
    c1w = nc.dram_tensor("c1w", [C, 1536], f32, kind="ExternalInput")
    c2w = nc.dram_tensor("c2w", [C, 128], f32, kind="ExternalInput")
    ctw = nc.dram_tensor("ctw", [C, 384], f32, kind="ExternalInput")
    c3w = nc.dram_tensor("c3w", [C, 128], f32, kind="ExternalInput")
    bvs = nc.dram_tensor("bvs", [C, 4], f32, kind="ExternalInput")
    zpd = nc.dram_tensor("zpd", [C, 2], f32, kind="ExternalInput")
    out = nc.dram_tensor("out", [BL, ND], f32, kind="ExternalOutput")
    lrscr = nc.dram_tensor("lrscr", [BL, 2, N], f32, kind="Internal")

    with tile.TileContext(nc) as tc, ExitStack() as ctx:
        cpool = ctx.enter_context(tc.tile_pool(name="consts", bufs=1))
        sigp = ctx.enter_context(tc.tile_pool(name="sigp", bufs=6))
        h1p = ctx.enter_context(tc.tile_pool(name="h1p", bufs=3))
        h3p = ctx.enter_context(tc.tile_pool(name="h3p", bufs=3))
        bigp = ctx.enter_context(tc.tile_pool(name="bigp", bufs=1))
        tailp = ctx.enter_context(tc.tile_pool(name="tailp", bufs=1))
        ps1p = ctx.enter_context(tc.tile_pool(name="ps1", bufs=3 if ps_rebal else 2, space="PSUM"))
        ps2p = ctx.enter_context(tc.tile_pool(name="ps2", bufs=2, space="PSUM"))
        ps3p = ctx.enter_context(tc.tile_pool(name="ps3", bufs=2, space="PSUM"))
        ps4p = ctx.enter_context(tc.tile_pool(name="ps4", bufs=1 if ps_rebal else 2, space="PSUM"))

        # constants (loaded once, outside any benchmark loop)
        c1w_t = cpool.tile([C, 1536], f32)
        nc.sync.dma_start(r(c1w_t[:]), c1w.ap().bitcast(f32r))
        c2w_t = cpool.tile([C, 128], f32)
        nc.sync.dma_start(r(c2w_t[:]), c2w.ap().bitcast(f32r))
        ctw_t = cpool.tile([C, 384], f32)
        nc.sync.dma_start(r(ctw_t[:]), ctw.ap().bitcast(f32r))
        c3w_t = cpool.tile([C, 128], f32)
        nc.sync.dma_start(r(c3w_t[:]), c3w.ap().bitcast(f32r))
        bvs_t = cpool.tile([C, 4], f32)
        nc.sync.dma_start(bvs_t[:], bvs.ap())
        cd0r_t = cpool.tile([PR, BL * CH], f32)
        nc.sync.dma_start(cd0r_t[:], cd0r.ap())
        cd1r_t = cpool.tile([PR, BL * CH], f32)
        nc.sync.dma_start(cd1r_t[:], cd1r.ap())

        # full-length stage tensors
        h2p = bigp.tile([128, ND + 4], f32, tag="h2p")   # h2[t] at col 2+t
        lrsb = bigp.tile([128, N], f32, tag="lrsb")
        nc.sync.dma_start(r(h2p[:, 0:2]), zpd.ap().bitcast(f32r))
        nc.sync.dma_start(r(h2p[:, ND + 2:ND + 4]), zpd.ap().bitcast(f32r))

        def tail_half(p0, p1):
            """mass/log tail for partitions [p0, p1) of the [PR, BL*CH]
            time-chunked layout (partition p covers t in [130p, 130p+130))."""
            nP = p1 - p0

            def shifted(off):
                t = tailp.tile([PR, BL * CH], f32, tag=f"sh{off}")
                src = bass.AP(lrscr, off + p0 * CH,
                              [[CH, nP], [2 * N, BL], [1, CH]])
                nc.sync.dma_start(
                    t[p0:p1].rearrange("p (b c) -> p b c", b=BL), src)
                return t

            sL1 = shifted(1)
            sL2 = shifted(2)
            sR0 = shifted(N)
            sR1 = shifted(N + 1)

            m1 = tailp.tile([PR, BL * CH], f32, tag="m1")
            nc.vector.tensor_mul(m1[p0:p1], cd1r_t[p0:p1], sR1[p0:p1])
            m2 = tailp.tile([PR, BL * CH], f32, tag="m2")
            nc.vector.tensor_mul(m2[p0:p1], cd0r_t[p0:p1], sL1[p0:p1])
            mi = tailp.tile([PR, BL * CH], f32, tag="mi")
            nc.vector.tensor_add(mi[p0:p1], m1[p0:p1], m2[p0:p1])
            mo = tailp.tile([PR, BL * CH], f32, tag="mo")
            nc.vector.tensor_add(mo[p0:p1], sR0[p0:p1], sL2[p0:p1])
            li = tailp.tile([PR, BL * CH], f32, tag="li")
            nc.scalar.activation(li[p0:p1], mi[p0:p1], AF.Ln)
            lo = tailp.tile([PR, BL * CH], f32, tag="lo")
            nc.scalar.activation(lo[p0:p1], mo[p0:p1], AF.Ln)
            ndt = tailp.tile([PR, BL * CH], f32, tag="ndt")
            nc.vector.tensor_sub(ndt[p0:p1], li[p0:p1], lo[p0:p1])

            dst = bass.AP(out, p0 * CH, [[CH, nP], [ND, BL], [1, CH]])
            nc.sync.dma_start(dst,
                              ndt[p0:p1].rearrange("p (b c) -> p b c", b=BL))

        def body():
            # fused pipelined loop: iteration i runs stage-A (conv1/relu1/
            # conv2/relu2) on tile i and stage-B (convT/relu3/conv3/sigmoid
            # + lrscr spill) on tile i-1, so stage-B engine work overlaps
            # the DMA-bound stage-A phase.
            for i in range(NT + 1):
                if i < NT:
                    t0 = i * TW
                    W = min(TW, ND - t0)
                    SW = TW + 2
                    sg = sigp.tile([C, BL * SW], f32, tag="sg")
                    if sig_batch:
                        src = bass.AP(sig, t0,
                                      [[N, C], [C * N, BL], [1, W + 2]])
                        nc.sync.dma_start(
                            r(sg[:]).rearrange("p (b c) -> p b c",
                                               b=BL)[:, :, :W + 2],
                            src.bitcast(f32r))
                    else:
                        for b in range(BL):
                            nc.sync.dma_start(
                                r(sg[:, b * SW:b * SW + W + 2]),
                                sig.ap()[b][:, t0:t0 + W + 2].bitcast(f32r))
                    p1 = ps1p.tile([128, TW], f32)
                    for k in range(3):
                        for b in range(BL):
                            m = 4 * k + b
                            nc.tensor.matmul(
                                p1[:, :W],
                                r(c1w_t[:, 128 * m:128 * m + 128]),
                                r(sg[:, b * SW + k:b * SW + k + W]),
                                start=(m == 0), stop=(m == 11))
                    h1f = h1p.tile([128, TW], f32, tag="h1f")
                    nc.scalar.activation(r(h1f[:, :W]), p1[:, :W],
                                         AF.Relu, bias=bvs_t[:, 0:1])

                    p2 = ps2p.tile([128, TW], f32)
                    nc.tensor.matmul(p2[:, :W], r(c2w_t[:]),
                                     r(h1f[:, :W]),
                                     start=True, stop=True)
                    nc.vector.tensor_scalar(r(h2p[:, 2 + t0:2 + t0 + W]),
                                            p2[:, :W], bvs_t[:, 1:2], 0.0,
                                            op0=ALU.add, op1=ALU.max)

                if i >= 1:
                    j = i - 1
                    t0 = j * TW
                    p3 = ps3p.tile([128, TW], f32)
                    for k in range(3):
                        nc.tensor.matmul(
                            p3[:], r(ctw_t[:, 128 * k:128 * k + 128]),
                            r(h2p[:, 2 + t0 - k:2 + t0 - k + TW]),
                            start=(k == 0), stop=(k == 2))
                    h3f = h3p.tile([128, TW], f32, tag="h3f")
                    nc.scalar.activation(r(h3f[:]), p3[:],
                                         AF.Relu, bias=bvs_t[:, 2:3])

                    p4 = ps4p.tile([128, TW], f32)
                    nc.tensor.matmul(p4[:], r(c3w_t[:]), r(h3f[:]),
                                     start=True, stop=True)
                    nc.scalar.activation(lrsb[:, t0:t0 + TW], p4[:],
                                         AF.Sigmoid, bias=bvs_t[:, 3:4])
                    if i % 4 == 0:
                        # spill the last 4 sigmoid tiles (one DMA per batch)
                        c0 = (j - 3) * TW
                        for b in range(BL):
                            nc.gpsimd.dma_start(
                                lrscr.ap()[b][:, c0:c0 + 4 * TW],
                                lrsb[32 * b:32 * b + 2, c0:c0 + 4 * TW])
                        if i == 12 and tail_split:
                            # partitions 0..31 cover t < 32*130+2 = 4162,
                            # all spilled by chunks 1-3: run this half here
                            # so it (and the Ln table load) hides under the
                            # loop
                            tail_half(0, 32)

            # second half of the tail (needs the last spill chunk)
            if tail_split:
                tail_half(32, PR)
            else:
                tail_half(0, PR)

        if loop_n > 1:
            with tc.For_i(0, loop_n, 1):
                body()
        else:
            body()

    nc.compile()
    return nc


def prep_inputs(signal, curr_diag, w1, b1, w2, b2, wt, bt, w3, b3, const):
    """Host-side prep: per-core in_maps (shard batch, pack weights)."""
    f32 = np.float32
    signal = np.asarray(signal, dtype=f32)
    curr_diag = np.asarray(curr_diag, dtype=f32)
    w1 = np.asarray(w1, dtype=f32)
    w2 = np.asarray(w2, dtype=f32)
    wt = np.asarray(wt, dtype=f32)
    w3 = np.asarray(w3, dtype=f32)
    const = float(const)

    c1w = np.zeros((C, 1536), f32)
    ctw = np.zeros((C, 384), f32)
    c2w = np.zeros((C, 128), f32)
    c3w = np.zeros((C, 128), f32)
    for k in range(3):
        for b in range(BL):
            o = 128 * (4 * k + b) + 32 * b
            c1w[:, o:o + 10] = w1[:, :, k].T
            o2 = 128 * k + 32 * b
            ctw[32 * b:32 * b + 10, o2:o2 + 10] = wt[:, :, k]
    for b in range(BL):
        c2w[32 * b:32 * b + 10, 32 * b:32 * b + 10] = w2[:, :, 0].T
        c3w[32 * b:32 * b + 10, 32 * b:32 * b + 2] = w3[:, :, 0].T
    bvs = np.zeros((C, 4), f32)
    for vec, width, col in ((b1, 10, 0), (b2, 10, 1), (bt, 10, 2), (b3, 2, 3)):
        v = np.asarray(vec, dtype=f32)
        for b in range(BL):
            bvs[32 * b:32 * b + width, col] = v

    in_maps = []
    for c in range(N_CORES):
        cd = curr_diag[BL * c:BL * (c + 1)]            # [BL, N-1]
        cd0 = (const * cd[:, 0:ND]).reshape(BL, PR, CH)
        cd1 = (const * cd[:, 1:ND + 1]).reshape(BL, PR, CH)
        in_maps.append({
            "sig": np.ascontiguousarray(signal[BL * c:BL * (c + 1)]),
            "cd0r": np.ascontiguousarray(
                cd0.transpose(1, 0, 2).reshape(PR, BL * CH)),
            "cd1r": np.ascontiguousarray(
                cd1.transpose(1, 0, 2).reshape(PR, BL * CH)),
            "c1w": c1w, "c2w": c2w, "ctw": ctw, "c3w": c3w, "bvs": bvs,
            "zpd": np.zeros((C, 2), f32),
        })
    return in_maps


def kernel(signal, curr_diag, index_diag, w1, b1, w2, b2, wt, bt, w3, b3,
           const):
    assert int(index_diag) == 1, "kernel specialized for index_diag == 1"
    from concourse.bass_utils import run_bass_kernel_spmd

    if "nc" not in _prog_cache:
        _prog_cache["nc"] = build_program()
    nc = _prog_cache["nc"]

    in_maps = prep_inputs(signal, curr_diag, w1, b1, w2, b2, wt, bt,
                          w3, b3, const)
    res = run_bass_kernel_spmd(nc, in_maps, core_ids=list(range(N_CORES)))
    full = np.concatenate([res.results[c]["out"] for c in range(N_CORES)],
                          axis=0)
    full = full - full.mean(dtype=np.float64).astype(np.float32)
    return full.astype(np.float32)


# revision 33
# speedup vs baseline: 1.0799x; 1.0799x over previous
"""Trainium2 Bass kernel for the DLEM converter + diagonal-update model.

Per batch:
    h1 = relu(conv1d(signal[128ch -> 10ch], k=3))        # [10, 8190]
    h2 = relu(conv1d(h1, k=1))                           # [10, 8190]
    h3 = relu(conv_transpose1d(h2, k=3))                 # [10, 8192]
    lr = sigmoid(conv1d(h3[10ch -> 2ch], k=1))           # [2, 8192]
    mass_in  = cd[1:]*right[1:n-1] + cd[:-1]*left[1:n-1]
    mass_out = right[0:n-2] + left[2:n]
    nd = ln(const*mass_in) - ln(mass_out);  out = nd - mean(nd)

Sharding: data-parallel over batch, 4 batches per core on 8 cores.

Device layout: the 4 local batches live in partition strips at 0/32/64/96
(matmul col-groups are 32-aligned), so conv2/convT/conv3 are single
block-diagonal fp32r matmuls (K=128) covering all 4 batches, and conv1 is
12 accumulating fp32r matmuls per 512-wide tile (4 batches x 3 taps, each
with a zero-padded full-width weight block so every PSUM row is written
and junk partitions accumulate exact zeros).  Biases ride for free on the
ACT/DVE epilogues (relu/sigmoid bias, tensor_scalar add+max).  The
elementwise tail (mass/log) runs in a dense time-partitioned layout
[63, 4*130] reached by bouncing sigmoid rows through a DRAM scratch and
reading back with shifted strided APs, so those ops use 63 lanes instead
of 8.  The global mean subtraction happens on host after the gather.

fp32r (TF32-style) matmuls run at 1 PE column/cycle vs 4 for strict fp32;
measured end-to-end error vs the fp32 reference is ~2e-5.
"""

import numpy as np

N_CORES = 8
B, C, N = 32, 128, 8192
BL = B // N_CORES          # batches per core
ND = N - 2                 # output length per batch (index_diag == 1)
CH = 130                   # time-chunk per partition in the tail layout
PR = 63                    # partitions used in tail (63*130 == 8190)
NT = 16                    # 512-wide time tiles
TW = 512

_prog_cache = {}


def build_program(loop_n=1, sig_batch=False, ps_rebal=False, tail_split=False):
    """Build + compile the per-core Bass program.

    loop_n > 1 wraps the whole body in an on-device For_i loop (used only
    for benchmarking; the work is identical every iteration).
    """
    import concourse.bass as bass
    import concourse.tile as tile
    import concourse.mybir as mybir
    from concourse import bacc
    from contextlib import ExitStack

    f32 = mybir.dt.float32
    f32r = mybir.dt.float32r
    AF = mybir.ActivationFunctionType
    ALU = mybir.AluOpType

    def r(ap):
        return ap.bitcast(f32r)

    nc = bacc.Bacc("TRN2", target_bir_lowering=False, debug=False,
                   num_devices=N_CORES)

    sig = nc.dram_tensor("sig", [BL, C, N], f32, kind="ExternalInput")
    cd0r = nc.dram_tensor("cd0r", [PR, BL * CH], f32, kind="ExternalInput")
    cd1r = nc.dram_tensor("cd1r", [PR, BL * CH], f32, kind="ExternalInput")
    c1w = nc.dram_tensor("c1w", [C, 1536], f32, kind="ExternalInput")
    c2w = nc.dram_tensor("c2w", [C, 128], f32, kind="ExternalInput")
    ctw = nc.dram_tensor("ctw", [C, 384], f32, kind="ExternalInput")
    c3w = nc.dram_tensor("c3w", [C, 128], f32, kind="ExternalInput")
    bvs = nc.dram_tensor("bvs", [C, 4], f32, kind="ExternalInput")
    zpd = nc.dram_tensor("zpd", [C, 2], f32, kind="ExternalInput")
    out = nc.dram_tensor("out", [BL, ND], f32, kind="ExternalOutput")
    lrscr = nc.dram_tensor("lrscr", [BL, 2, N], f32, kind="Internal")

    with tile.TileContext(nc) as tc, ExitStack() as ctx:
        cpool = ctx.enter_context(tc.tile_pool(name="consts", bufs=1))
        sigp = ctx.enter_context(tc.tile_pool(name="sigp", bufs=6))
        h1p = ctx.enter_context(tc.tile_pool(name="h1p", bufs=3))
        h3p = ctx.enter_context(tc.tile_pool(name="h3p", bufs=3))
        bigp = ctx.enter_context(tc.tile_pool(name="bigp", bufs=1))
        tailp = ctx.enter_context(tc.tile_pool(name="tailp", bufs=1))
        ps1p = ctx.enter_context(tc.tile_pool(name="ps1", bufs=3 if ps_rebal else 2, space="PSUM"))
        ps2p = ctx.enter_context(tc.tile_pool(name="ps2", bufs=2, space="PSUM"))
        ps3p = ctx.enter_context(tc.tile_pool(name="ps3", bufs=2, space="PSUM"))
        ps4p = ctx.enter_context(tc.tile_pool(name="ps4", bufs=1 if ps_rebal else 2, space="PSUM"))

        # constants (loaded once, outside any benchmark loop)
        c1w_t = cpool.tile([C, 1536], f32)
        nc.sync.dma_start(r(c1w_t[:]), c1w.ap().bitcast(f32r))
        c2w_t = cpool.tile([C, 128], f32)
        nc.sync.dma_start(r(c2w_t[:]), c2w.ap().bitcast(f32r))
        ctw_t = cpool.tile([C, 384], f32)
        nc.sync.dma_start(r(ctw_t[:]), ctw.ap().bitcast(f32r))
        c3w_t = cpool.tile([C, 128], f32)
        nc.sync.dma_start(r(c3w_t[:]), c3w.ap().bitcast(f32r))
        bvs_t = cpool.tile([C, 4], f32)
        nc.sync.dma_start(bvs_t[:], bvs.ap())
        cd0r_t = cpool.tile([PR, BL * CH], f32)
        nc.sync.dma_start(cd0r_t[:], cd0r.ap())
        cd1r_t = cpool.tile([PR, BL * CH], f32)
        nc.sync.dma_start(cd1r_t[:], cd1r.ap())

        # full-length stage tensors
        h2p = bigp.tile([128, ND + 4], f32, tag="h2p")   # h2[t] at col 2+t
        lrsb = bigp.tile([128, N], f32, tag="lrsb")
        nc.sync.dma_start(r(h2p[:, 0:2]), zpd.ap().bitcast(f32r))
        nc.sync.dma_start(r(h2p[:, ND + 2:ND + 4]), zpd.ap().bitcast(f32r))

        def tail_half(p0, p1):
            """mass/log tail for partitions [p0, p1) of the [PR, BL*CH]
            time-chunked layout (partition p covers t in [130p, 130p+130))."""
            nP = p1 - p0

            def shifted(off):
                t = tailp.tile([PR, BL * CH], f32, tag=f"sh{off}")
                src = bass.AP(lrscr, off + p0 * CH,
                              [[CH, nP], [2 * N, BL], [1, CH]])
                nc.sync.dma_start(
                    t[p0:p1].rearrange("p (b c) -> p b c", b=BL), src)
                return t

            sL1 = shifted(1)
            sL2 = shifted(2)
            sR0 = shifted(N)
            sR1 = shifted(N + 1)

            m1 = tailp.tile([PR, BL * CH], f32, tag="m1")
            nc.vector.tensor_mul(m1[p0:p1], cd1r_t[p0:p1], sR1[p0:p1])
            m2 = tailp.tile([PR, BL * CH], f32, tag="m2")
            nc.vector.tensor_mul(m2[p0:p1], cd0r_t[p0:p1], sL1[p0:p1])
            mi = tailp.tile([PR, BL * CH], f32, tag="mi")
            nc.vector.tensor_add(mi[p0:p1], m1[p0:p1], m2[p0:p1])
            mo = tailp.tile([PR, BL * CH], f32, tag="mo")
            nc.vector.tensor_add(mo[p0:p1], sR0[p0:p1], sL2[p0:p1])
            li = tailp.tile([PR, BL * CH], f32, tag="li")
            nc.scalar.activation(li[p0:p1], mi[p0:p1], AF.Ln)
            lo = tailp.tile([PR, BL * CH], f32, tag="lo")
            nc.scalar.activation(lo[p0:p1], mo[p0:p1], AF.Ln)
            ndt = tailp.tile([PR, BL * CH], f32, tag="ndt")
            nc.vector.tensor_sub(ndt[p0:p1], li[p0:p1], lo[p0:p1])

            dst = bass.AP(out, p0 * CH, [[CH, nP], [ND, BL], [1, CH]])
            nc.sync.dma_start(dst,
                              ndt[p0:p1].rearrange("p (b c) -> p b c", b=BL))

        def body():
            # fused pipelined loop: iteration i runs stage-A (conv1/relu1/
            # conv2/relu2) on tile i and stage-B (convT/relu3/conv3/sigmoid
            # + lrscr spill) on tile i-1, so stage-B engine work overlaps
            # the DMA-bound stage-A phase.
            for i in range(NT + 1):
                if i < NT:
                    t0 = i * TW
                    W = min(TW, ND - t0)
                    SW = TW + 2
                    sg = sigp.tile([C, BL * SW], f32, tag="sg")
                    if sig_batch:
                        src = bass.AP(sig, t0,
                                      [[N, C], [C * N, BL], [1, W + 2]])
                        nc.sync.dma_start(
                            r(sg[:]).rearrange("p (b c) -> p b c",
                                               b=BL)[:, :, :W + 2],
                            src.bitcast(f32r))
                    else:
                        for b in range(BL):
                            nc.sync.dma_start(
                                r(sg[:, b * SW:b * SW + W + 2]),
                                sig.ap()[b][:, t0:t0 + W + 2].bitcast(f32r))
                    p1 = ps1p.tile([128, TW], f32)
                    for k in range(3):
                        for b in range(BL):
                            m = 4 * k + b
                            nc.tensor.matmul(
                                p1[:, :W],
                                r(c1w_t[:, 128 * m:128 * m + 128]),
                                r(sg[:, b * SW + k:b * SW + k + W]),
                                start=(m == 0), stop=(m == 11))
                    h1f = h1p.tile([128, TW], f32, tag="h1f")
                    nc.scalar.activation(r(h1f[:, :W]), p1[:, :W],
                                         AF.Relu, bias=bvs_t[:, 0:1])

                    p2 = ps2p.tile([128, TW], f32)
                    nc.tensor.matmul(p2[:, :W], r(c2w_t[:]),
                                     r(h1f[:, :W]),
                                     start=True, stop=True)
                    nc.vector.tensor_scalar(r(h2p[:, 2 + t0:2 + t0 + W]),
                                            p2[:, :W], bvs_t[:, 1:2], 0.0,
                                            op0=ALU.add, op1=ALU.max)

                if i >= 1:
                    j = i - 1
                    t0 = j * TW
                    p3 = ps3p.tile([128, TW], f32)
                    for k in range(3):
                        nc.tensor.matmul(
                            p3[:], r(ctw_t[:, 128 * k:128 * k + 128]),
                            r(h2p[:, 2 + t0 - k:2 + t0 - k + TW]),
                            start=(k == 0), stop=(k == 2))
                    h3f = h3p.tile([128, TW], f32, tag="h3f")
                    nc.scalar.activation(r(h3f[:]), p3[:],
                                         AF.Relu, bias=bvs_t[:, 2:3])

                    p4 = ps4p.tile([128, TW], f32)
                    nc.tensor.matmul(p4[:], r(c3w_t[:]), r(h3f[:]),
                                     start=True, stop=True)
                    nc.scalar.activation(lrsb[:, t0:t0 + TW], p4[:],
                                         AF.Sigmoid, bias=bvs_t[:, 3:4])
                    if i % 4 == 0:
                        # spill the last 4 sigmoid tiles (one DMA per batch)
                        c0 = (j - 3) * TW
                        for b in range(BL):
                            nc.gpsimd.dma_start(
                                lrscr.ap()[b][:, c0:c0 + 4 * TW],
                                lrsb[32 * b:32 * b + 2, c0:c0 + 4 * TW])
                        if i == 12 and tail_split:
                            # partitions 0..31 cover t < 32*130+2 = 4162,
                            # all spilled by chunks 1-3: run this half here
                            # so it (and the Ln table load) hides under the
                            # loop
                            tail_half(0, 32)

            # second half of the tail (needs the last spill chunk)
            if tail_split:
                tail_half(32, PR)
            else:
                tail_half(0, PR)

        if loop_n > 1:
            with tc.For_i(0, loop_n, 1):
                body()
        else:
            body()

    nc.compile()
    return nc


def prep_inputs(signal, curr_diag, w1, b1, w2, b2, wt, bt, w3, b3, const):
    """Host-side prep: per-core in_maps (shard batch, pack weights)."""
    f32 = np.float32
    signal = np.asarray(signal, dtype=f32)
    curr_diag = np.asarray(curr_diag, dtype=f32)
    w1 = np.asarray(w1, dtype=f32)
    w2 = np.asarray(w2, dtype=f32)
    wt = np.asarray(wt, dtype=f32)
    w3 = np.asarray(w3, dtype=f32)
    const = float(const)

    c1w = np.zeros((C, 1536), f32)
    ctw = np.zeros((C, 384), f32)
    c2w = np.zeros((C, 128), f32)
    c3w = np.zeros((C, 128), f32)
    for k in range(3):
        for b in range(BL):
            o = 128 * (4 * k + b) + 32 * b
            c1w[:, o:o + 10] = w1[:, :, k].T
            o2 = 128 * k + 32 * b
            ctw[32 * b:32 * b + 10, o2:o2 + 10] = wt[:, :, k]
    for b in range(BL):
        c2w[32 * b:32 * b + 10, 32 * b:32 * b + 10] = w2[:, :, 0].T
        c3w[32 * b:32 * b + 10, 32 * b:32 * b + 2] = w3[:, :, 0].T
    bvs = np.zeros((C, 4), f32)
    for vec, width, col in ((b1, 10, 0), (b2, 10, 1), (bt, 10, 2), (b3, 2, 3)):
        v = np.asarray(vec, dtype=f32)
        for b in range(BL):
            bvs[32 * b:32 * b + width, col] = v

    in_maps = []
    for c in range(N_CORES):
        cd = curr_diag[BL * c:BL * (c + 1)]            # [BL, N-1]
        cd0 = (const * cd[:, 0:ND]).reshape(BL, PR, CH)
        cd1 = (const * cd[:, 1:ND + 1]).reshape(BL, PR, CH)
        in_maps.append({
            "sig": np.ascontiguousarray(signal[BL * c:BL * (c + 1)]),
            "cd0r": np.ascontiguousarray(
                cd0.transpose(1, 0, 2).reshape(PR, BL * CH)),
            "cd1r": np.ascontiguousarray(
                cd1.transpose(1, 0, 2).reshape(PR, BL * CH)),
            "c1w": c1w, "c2w": c2w, "ctw": ctw, "c3w": c3w, "bvs": bvs,
            "zpd": np.zeros((C, 2), f32),
        })
    return in_maps


def kernel(signal, curr_diag, index_diag, w1, b1, w2, b2, wt, bt, w3, b3,
           const):
    assert int(index_diag) == 1, "kernel specialized for index_diag == 1"
    from concourse.bass_utils import run_bass_kernel_spmd

    if "nc" not in _prog_cache:
        _prog_cache["nc"] = build_program()
    nc = _prog_cache["nc"]

    in_maps = prep_inputs(signal, curr_diag, w1, b1, w2, b2, wt, bt,
                          w3, b3, const)
    res = run_bass_kernel_spmd(nc, in_maps, core_ids=list(range(N_CORES)))
    full = np.concatenate([res.results[c]["out"] for c in range(N_CORES)],
                          axis=0)
    full = full - full.mean(dtype=np.float64).astype(np.float32)
    return full.astype(np.float32)


# revision 34
# speedup vs baseline: 1.0921x; 1.0113x over previous
"""Trainium2 Bass kernel for the DLEM converter + diagonal-update model.

Per batch:
    h1 = relu(conv1d(signal[128ch -> 10ch], k=3))        # [10, 8190]
    h2 = relu(conv1d(h1, k=1))                           # [10, 8190]
    h3 = relu(conv_transpose1d(h2, k=3))                 # [10, 8192]
    lr = sigmoid(conv1d(h3[10ch -> 2ch], k=1))           # [2, 8192]
    mass_in  = cd[1:]*right[1:n-1] + cd[:-1]*left[1:n-1]
    mass_out = right[0:n-2] + left[2:n]
    nd = ln(const*mass_in) - ln(mass_out);  out = nd - mean(nd)

Sharding: data-parallel over batch, 4 batches per core on 8 cores.

Device layout: the 4 local batches live in partition strips at 0/32/64/96
(matmul col-groups are 32-aligned), so conv2/convT/conv3 are single
block-diagonal fp32r matmuls (K=128) covering all 4 batches, and conv1 is
12 accumulating fp32r matmuls per 512-wide tile (4 batches x 3 taps, each
with a zero-padded full-width weight block so every PSUM row is written
and junk partitions accumulate exact zeros).  Biases ride for free on the
ACT/DVE epilogues (relu/sigmoid bias, tensor_scalar add+max).  The
elementwise tail (mass/log) runs in a dense time-partitioned layout
[63, 4*130] reached by bouncing sigmoid rows through a DRAM scratch and
reading back with shifted strided APs, so those ops use 63 lanes instead
of 8.  The global mean subtraction happens on host after the gather.

fp32r (TF32-style) matmuls run at 1 PE column/cycle vs 4 for strict fp32;
measured end-to-end error vs the fp32 reference is ~2e-5.
"""

import numpy as np

N_CORES = 8
B, C, N = 32, 128, 8192
BL = B // N_CORES          # batches per core
ND = N - 2                 # output length per batch (index_diag == 1)
CH = 130                   # time-chunk per partition in the tail layout
PR = 63                    # partitions used in tail (63*130 == 8190)
NT = 16                    # 512-wide time tiles
TW = 512

_prog_cache = {}


def build_program(loop_n=1, sig_batch=False, ps_rebal=False, tail_split=False):
    """Build + compile the per-core Bass program.

    loop_n > 1 wraps the whole body in an on-device For_i loop (used only
    for benchmarking; the work is identical every iteration).
    """
    import concourse.bass as bass
    import concourse.tile as tile
    import concourse.mybir as mybir
    from concourse import bacc
    from contextlib import ExitStack

    f32 = mybir.dt.float32
    f32r = mybir.dt.float32r
    AF = mybir.ActivationFunctionType
    ALU = mybir.AluOpType

    def r(ap):
        return ap.bitcast(f32r)

    nc = bacc.Bacc("TRN2", target_bir_lowering=False, debug=False,
                   num_devices=N_CORES)

    sig = nc.dram_tensor("sig", [BL, C, N], f32, kind="ExternalInput")
    cd0r = nc.dram_tensor("cd0r", [PR, BL * CH], f32, kind="ExternalInput")
    cd1r = nc.dram_tensor("cd1r", [PR, BL * CH], f32, kind="ExternalInput")
    c1w = nc.dram_tensor("c1w", [C, 1536], f32, kind="ExternalInput")
    c2w = nc.dram_tensor("c2w", [C, 128], f32, kind="ExternalInput")
    ctw = nc.dram_tensor("ctw", [C, 384], f32, kind="ExternalInput")
    c3w = nc.dram_tensor("c3w", [C, 128], f32, kind="ExternalInput")
    bvs = nc.dram_tensor("bvs", [C, 4], f32, kind="ExternalInput")
    zpd = nc.dram_tensor("zpd", [C, 2], f32, kind="ExternalInput")
    out = nc.dram_tensor("out", [BL, ND], f32, kind="ExternalOutput")
    lrscr = nc.dram_tensor("lrscr", [BL, 2, N], f32, kind="Internal")

    with tile.TileContext(nc) as tc, ExitStack() as ctx:
        cpool = ctx.enter_context(tc.tile_pool(name="consts", bufs=1))
        sigp = ctx.enter_context(tc.tile_pool(name="sigp", bufs=6))
        h1p = ctx.enter_context(tc.tile_pool(name="h1p", bufs=3))
        h3p = ctx.enter_context(tc.tile_pool(name="h3p", bufs=3))
        bigp = ctx.enter_context(tc.tile_pool(name="bigp", bufs=1))
        tailp = ctx.enter_context(tc.tile_pool(name="tailp", bufs=1))
        ps1p = ctx.enter_context(tc.tile_pool(name="ps1", bufs=3 if ps_rebal else 2, space="PSUM"))
        ps2p = ctx.enter_context(tc.tile_pool(name="ps2", bufs=2, space="PSUM"))
        ps3p = ctx.enter_context(tc.tile_pool(name="ps3", bufs=2, space="PSUM"))
        ps4p = ctx.enter_context(tc.tile_pool(name="ps4", bufs=1 if ps_rebal else 2, space="PSUM"))

        # constants (loaded once, outside any benchmark loop)
        c1w_t = cpool.tile([C, 1536], f32)
        nc.sync.dma_start(r(c1w_t[:]), c1w.ap().bitcast(f32r))
        c2w_t = cpool.tile([C, 128], f32)
        nc.sync.dma_start(r(c2w_t[:]), c2w.ap().bitcast(f32r))
        ctw_t = cpool.tile([C, 384], f32)
        nc.sync.dma_start(r(ctw_t[:]), ctw.ap().bitcast(f32r))
        c3w_t = cpool.tile([C, 128], f32)
        nc.sync.dma_start(r(c3w_t[:]), c3w.ap().bitcast(f32r))
        bvs_t = cpool.tile([C, 4], f32)
        nc.sync.dma_start(bvs_t[:], bvs.ap())
        cd0r_t = cpool.tile([PR, BL * CH], f32)
        nc.sync.dma_start(cd0r_t[:], cd0r.ap())
        cd1r_t = cpool.tile([PR, BL * CH], f32)
        nc.sync.dma_start(cd1r_t[:], cd1r.ap())

        # full-length stage tensors
        h2p = bigp.tile([128, ND + 4], f32, tag="h2p")   # h2[t] at col 2+t
        lrsb = bigp.tile([128, N], f32, tag="lrsb")
        nc.sync.dma_start(r(h2p[:, 0:2]), zpd.ap().bitcast(f32r))
        nc.sync.dma_start(r(h2p[:, ND + 2:ND + 4]), zpd.ap().bitcast(f32r))

        def tail_half(p0, p1):
            """mass/log tail for partitions [p0, p1) of the [PR, BL*CH]
            time-chunked layout (partition p covers t in [130p, 130p+130))."""
            nP = p1 - p0

            def shifted(off):
                t = tailp.tile([PR, BL * CH], f32, tag=f"sh{off}")
                src = bass.AP(lrscr, off + p0 * CH,
                              [[CH, nP], [2 * N, BL], [1, CH]])
                nc.sync.dma_start(
                    t[p0:p1].rearrange("p (b c) -> p b c", b=BL), src)
                return t

            sL1 = shifted(1)
            sL2 = shifted(2)
            sR0 = shifted(N)
            sR1 = shifted(N + 1)

            m1 = tailp.tile([PR, BL * CH], f32, tag="m1")
            nc.vector.tensor_mul(m1[p0:p1], cd1r_t[p0:p1], sR1[p0:p1])
            m2 = tailp.tile([PR, BL * CH], f32, tag="m2")
            nc.vector.tensor_mul(m2[p0:p1], cd0r_t[p0:p1], sL1[p0:p1])
            mi = tailp.tile([PR, BL * CH], f32, tag="mi")
            nc.vector.tensor_add(mi[p0:p1], m1[p0:p1], m2[p0:p1])
            mo = tailp.tile([PR, BL * CH], f32, tag="mo")
            nc.vector.tensor_add(mo[p0:p1], sR0[p0:p1], sL2[p0:p1])
            li = tailp.tile([PR, BL * CH], f32, tag="li")
            nc.scalar.activation(li[p0:p1], mi[p0:p1], AF.Ln)
            lo = tailp.tile([PR, BL * CH], f32, tag="lo")
            nc.scalar.activation(lo[p0:p1], mo[p0:p1], AF.Ln)
            ndt = tailp.tile([PR, BL * CH], f32, tag="ndt")
            nc.vector.tensor_sub(ndt[p0:p1], li[p0:p1], lo[p0:p1])

            dst = bass.AP(out, p0 * CH, [[CH, nP], [ND, BL], [1, CH]])
            nc.sync.dma_start(dst,
                              ndt[p0:p1].rearrange("p (b c) -> p b c", b=BL))

        def body():
            # fused pipelined loop: iteration i runs stage-A (conv1/relu1/
            # conv2/relu2) on tile i and stage-B (convT/relu3/conv3/sigmoid
            # + lrscr spill) on tile i-1, so stage-B engine work overlaps
            # the DMA-bound stage-A phase.
            for i in range(NT + 1):
                if i < NT:
                    t0 = i * TW
                    W = min(TW, ND - t0)
                    SW = TW + 2
                    sg = sigp.tile([C, BL * SW], f32, tag="sg")
                    if sig_batch:
                        src = bass.AP(sig, t0,
                                      [[N, C], [C * N, BL], [1, W + 2]])
                        nc.sync.dma_start(
                            r(sg[:]).rearrange("p (b c) -> p b c",
                                               b=BL)[:, :, :W + 2],
                            src.bitcast(f32r))
                    else:
                        for b in range(BL):
                            nc.sync.dma_start(
                                r(sg[:, b * SW:b * SW + W + 2]),
                                sig.ap()[b][:, t0:t0 + W + 2].bitcast(f32r))
                    p1 = ps1p.tile([128, TW], f32)
                    for k in range(3):
                        for b in range(BL):
                            m = 4 * k + b
                            nc.tensor.matmul(
                                p1[:, :W],
                                r(c1w_t[:, 128 * m:128 * m + 128]),
                                r(sg[:, b * SW + k:b * SW + k + W]),
                                start=(m == 0), stop=(m == 11))
                    h1f = h1p.tile([128, TW], f32, tag="h1f")
                    nc.scalar.activation(r(h1f[:, :W]), p1[:, :W],
                                         AF.Relu, bias=bvs_t[:, 0:1])

                    p2 = ps2p.tile([128, TW], f32)
                    nc.tensor.matmul(p2[:, :W], r(c2w_t[:]),
                                     r(h1f[:, :W]),
                                     start=True, stop=True)
                    nc.vector.tensor_scalar(r(h2p[:, 2 + t0:2 + t0 + W]),
                                            p2[:, :W], bvs_t[:, 1:2], 0.0,
                                            op0=ALU.add, op1=ALU.max)

                if i >= 1:
                    j = i - 1
                    t0 = j * TW
                    p3 = ps3p.tile([128, TW], f32)
                    for k in range(3):
                        nc.tensor.matmul(
                            p3[:], r(ctw_t[:, 128 * k:128 * k + 128]),
                            r(h2p[:, 2 + t0 - k:2 + t0 - k + TW]),
                            start=(k == 0), stop=(k == 2))
                    h3f = h3p.tile([128, TW], f32, tag="h3f")
                    nc.scalar.activation(r(h3f[:]), p3[:],
                                         AF.Relu, bias=bvs_t[:, 2:3])

                    p4 = ps4p.tile([128, TW], f32)
                    nc.tensor.matmul(p4[:], r(c3w_t[:]), r(h3f[:]),
                                     start=True, stop=True)
                    nc.scalar.activation(lrsb[:, t0:t0 + TW], p4[:],
                                         AF.Sigmoid, bias=bvs_t[:, 3:4])
                    if i % 4 == 0:
                        # spill the last 4 sigmoid tiles (one DMA per batch)
                        c0 = (j - 3) * TW
                        for b in range(BL):
                            nc.gpsimd.dma_start(
                                lrscr.ap()[b][:, c0:c0 + 4 * TW],
                                lrsb[32 * b:32 * b + 2, c0:c0 + 4 * TW])
                        if i == 12 and tail_split:
                            # partitions 0..31 cover t < 32*130+2 = 4162,
                            # all spilled by chunks 1-3: run this half here
                            # so it (and the Ln table load) hides under the
                            # loop
                            tail_half(0, 32)

            # second half of the tail (needs the last spill chunk)
            if tail_split:
                tail_half(32, PR)
            else:
                tail_half(0, PR)

        if loop_n > 1:
            with tc.For_i(0, loop_n, 1):
                body()
        else:
            body()

    nc.compile()
    return nc


def prep_inputs(signal, curr_diag, w1, b1, w2, b2, wt, bt, w3, b3, const):
    """Host-side prep: per-core in_maps (shard batch, pack weights)."""
    f32 = np.float32
    signal = np.asarray(signal, dtype=f32)
    curr_diag = np.asarray(curr_diag, dtype=f32)
    w1 = np.asarray(w1, dtype=f32)
    w2 = np.asarray(w2, dtype=f32)
    wt = np.asarray(wt, dtype=f32)
    w3 = np.asarray(w3, dtype=f32)
    const = float(const)

    c1w = np.zeros((C, 1536), f32)
    ctw = np.zeros((C, 384), f32)
    c2w = np.zeros((C, 128), f32)
    c3w = np.zeros((C, 128), f32)
    for k in range(3):
        for b in range(BL):
            o = 128 * (4 * k + b) + 32 * b
            c1w[:, o:o + 10] = w1[:, :, k].T
            o2 = 128 * k + 32 * b
            ctw[32 * b:32 * b + 10, o2:o2 + 10] = wt[:, :, k]
    for b in range(BL):
        c2w[32 * b:32 * b + 10, 32 * b:32 * b + 10] = w2[:, :, 0].T
        c3w[32 * b:32 * b + 10, 32 * b:32 * b + 2] = w3[:, :, 0].T
    bvs = np.zeros((C, 4), f32)
    for vec, width, col in ((b1, 10, 0), (b2, 10, 1), (bt, 10, 2), (b3, 2, 3)):
        v = np.asarray(vec, dtype=f32)
        for b in range(BL):
            bvs[32 * b:32 * b + width, col] = v

    in_maps = []
    for c in range(N_CORES):
        cd = curr_diag[BL * c:BL * (c + 1)]            # [BL, N-1]
        cd0 = (const * cd[:, 0:ND]).reshape(BL, PR, CH)
        cd1 = (const * cd[:, 1:ND + 1]).reshape(BL, PR, CH)
        in_maps.append({
            "sig": np.ascontiguousarray(signal[BL * c:BL * (c + 1)]),
            "cd0r": np.ascontiguousarray(
                cd0.transpose(1, 0, 2).reshape(PR, BL * CH)),
            "cd1r": np.ascontiguousarray(
                cd1.transpose(1, 0, 2).reshape(PR, BL * CH)),
            "c1w": c1w, "c2w": c2w, "ctw": ctw, "c3w": c3w, "bvs": bvs,
            "zpd": np.zeros((C, 2), f32),
        })
    return in_maps


def kernel(signal, curr_diag, index_diag, w1, b1, w2, b2, wt, bt, w3, b3,
           const):
    assert int(index_diag) == 1, "kernel specialized for index_diag == 1"
    assert tuple(np.shape(signal)) == (B, C, N), np.shape(signal)
    assert tuple(np.shape(curr_diag)) == (B, N - 1), np.shape(curr_diag)
    from concourse.bass_utils import run_bass_kernel_spmd

    if "nc" not in _prog_cache:
        _prog_cache["nc"] = build_program()
    nc = _prog_cache["nc"]

    in_maps = prep_inputs(signal, curr_diag, w1, b1, w2, b2, wt, bt,
                          w3, b3, const)
    res = run_bass_kernel_spmd(nc, in_maps, core_ids=list(range(N_CORES)))
    full = np.concatenate([res.results[c]["out"] for c in range(N_CORES)],
                          axis=0)
    full = full - full.mean(dtype=np.float64).astype(np.float32)
    return full.astype(np.float32)
